# revision 6
# baseline (speedup 1.0000x reference)
"""W4A16 group-quantized GEMM on 8 Trainium2 NeuronCores.

Problem: out[b,s,n] = x[b,s,:] @ dequant(W).T where W is INT4
group-quantized (group 128 along K), x is (4,4096,4096) fp16,
W is (11008, 4096) int4 + (11008, 32) fp16 scales.

Strategy (Megatron column-parallel): shard N=11008 -> 1376 per core,
replicate x. Per core: dequantize the weight shard once into SBUF
(resident, 88KB/partition as 32 k-tiles of [128, 1376] fp16), then
stream x.T through the PE accumulating over the 32 k-tiles in PSUM.

Host-side prep (layout only): x is transposed to (K, M) so K lands on
partitions; the int4 nibbles are re-paired so that one byte holds the
nibbles of (n, n+688) for the same k, giving the device a stride-free
unpack: low nibbles -> wT[:, 0:688], high -> wT[:, 688:1376]. Data
stays 4-bit-packed in DRAM; the device does the actual unpack+dequant.
"""

import sys

import numpy as np

if "/opt/trn_rl_repo" not in sys.path:
    sys.path.insert(0, "/opt/trn_rl_repo")

import concourse.bass as bass
import concourse.mybir as mybir
import concourse.tile as tile

def _split_multiwaits_json(bir_json: bytes) -> bytes:
    """Walrus in this environment encodes at most ONE sync-wait per
    instruction; Tile emits several. Split extras onto preceding same-engine
    NoOps (engine executes in order, so blocking semantics are identical)."""
    import orjson

    m = orjson.loads(bir_json)
    for fn in m.get("functions", []):
        for blk in fn.get("blocks", []):
            insts = blk.get("instructions")
            if not insts:
                continue
            out = []
            for ins in insts:
                si = ins.get("sync_info")
                if si:
                    ow = si.get("on_wait") or []
                    if len(ow) > 1:
                        for i, w in enumerate(ow[:-1]):
                            out.append(
                                {
                                    "debug": ins.get("debug", 0),
                                    "engine": ins["engine"],
                                    "ins": [],
                                    "outs": [],
                                    "name": f"{ins['name']}-sw{i}",
                                    "opcode": "NoOp",
                                    "sync_info": {"on_update": [], "on_wait": [w]},
                                }
                            )
                        si["on_wait"] = [ow[-1]]
                out.append(ins)
            blk["instructions"] = out
    return orjson.dumps(m)


def _install_walrus_compat_patch():
    from concourse import bass2jax as b2j
    from concourse import bass_utils as bu

    if getattr(bu.compile_bir_kernel, "_mw_patched", False):
        return
    orig = bu.compile_bir_kernel

    def patched(bir_json, tmpdir, neff_name="file.neff"):
        return orig(_split_multiwaits_json(bir_json), tmpdir, neff_name=neff_name)

    patched._mw_patched = True
    bu.compile_bir_kernel = patched
    b2j.compile_bir_kernel = patched


_install_walrus_compat_patch()

P = 128
K = 4096
N = 11008
M = 16384  # 4 * 4096 tokens
GROUP = 128
KG = K // GROUP  # 32 scale groups
NCORES = 8
NC = N // NCORES  # 1376 output cols per core
NHALF = NC // 2  # 688
KT = K // P  # 32 k-tiles
MB = 512  # m rows per x DMA block
CHUNKS = [(0, 512), (512, 1024), (1024, 1376)]  # psum n-chunks


def build_program(m_total: int = M, reps: int = 1) -> bass.Bass:
    """reps>1 duplicates the main GEMM loop (output overwritten each rep) —
    used only for differential timing of one epoch on hardware."""
    nc = bass.Bass()
    xT = nc.declare_dram_parameter(
        "xT", [K, m_total], mybir.dt.float16, isOutput=False
    )
    qT = nc.declare_dram_parameter("qT", [K, NHALF], mybir.dt.uint8, isOutput=False)
    sT = nc.declare_dram_parameter(
        "scalesT", [KG, NC], mybir.dt.float16, isOutput=False
    )
    out = nc.declare_dram_parameter(
        "out", [m_total, NC], mybir.dt.float16, isOutput=True
    )

    with tile.TileContext(nc) as tc:
        with (
            tc.tile_pool(name="wres", bufs=1) as wpool,
            tc.tile_pool(name="deq", bufs=2) as dqpool,
            tc.tile_pool(name="xin", bufs=2) as xpool,
            tc.tile_pool(name="outsb", bufs=3) as opool,
            tc.tile_pool(name="psA", bufs=2, space="PSUM") as psA,
            tc.tile_pool(name="psB", bufs=2, space="PSUM") as psB,
            tc.tile_pool(name="psC", bufs=2, space="PSUM") as psC,
        ):
            # ---- dequant W shard into resident SBUF wT [P, KT, NC] fp16 ----
            wT = wpool.tile([P, KT, NC], mybir.dt.float16)
            for t in range(KT):
                bt = dqpool.tile([P, NHALF], mybir.dt.uint8, tag="bytes")
                nc.sync.dma_start(bt[:], qT[t * P : (t + 1) * P, :])
                st = dqpool.tile([P, NC], mybir.dt.float16, tag="scale")
                nc.sync.dma_start(st[:], sT[t : t + 1, :].to_broadcast((P, NC)))
                # bitwise ops can't cast (u8->u8); the arith subtract casts
                qlo = dqpool.tile([P, NHALF], mybir.dt.uint8, tag="qlo")
                nc.vector.tensor_scalar(
                    out=qlo[:],
                    in0=bt[:],
                    scalar1=0x0F,
                    scalar2=None,
                    op0=mybir.AluOpType.bitwise_and,
                )
                qhi = dqpool.tile([P, NHALF], mybir.dt.uint8, tag="qhi")
                nc.vector.tensor_scalar(
                    out=qhi[:],
                    in0=bt[:],
                    scalar1=4,
                    scalar2=None,
                    op0=mybir.AluOpType.logical_shift_right,
                )
                nc.vector.tensor_scalar(
                    out=wT[:, t, 0:NHALF],
                    in0=qlo[:],
                    scalar1=8.0,
                    scalar2=None,
                    op0=mybir.AluOpType.subtract,
                )
                nc.vector.tensor_scalar(
                    out=wT[:, t, NHALF:NC],
                    in0=qhi[:],
                    scalar1=8.0,
                    scalar2=None,
                    op0=mybir.AluOpType.subtract,
                )
                nc.vector.tensor_tensor(
                    out=wT[:, t, :],
                    in0=wT[:, t, :],
                    in1=st[:],
                    op=mybir.AluOpType.mult,
                )

            # ---- main GEMM: out[m0:m0+128, :] = xT[:, m].T @ wT ----
            xview = xT.rearrange("(ko p) m -> p ko m", p=P)  # [128, KT, m_total]
            pools = [psA, psB, psC]
            n_blocks = m_total // MB
            for mb_r in range(n_blocks * reps):
                mb = mb_r % n_blocks
                xblk = xpool.tile([P, KT, MB], mybir.dt.float16, tag="xblk")
                for kc in range(8):  # split 4MB block across DMA queues
                    nc.sync.dma_start(
                        xblk[:, kc * 4 : (kc + 1) * 4, :],
                        xview[:, kc * 4 : (kc + 1) * 4, mb * MB : (mb + 1) * MB],
                    )
                for j in range(MB // P):
                    pss = [
                        pools[ci].tile(
                            [P, 512], mybir.dt.float32, name=f"ps{ci}"
                        )[:, : c1 - c0]
                        for ci, (c0, c1) in enumerate(CHUNKS)
                    ]
                    for t in range(KT):
                        lhsT = xblk[:, t, j * P : (j + 1) * P]
                        for ci, (c0, c1) in enumerate(CHUNKS):
                            nc.tensor.matmul(
                                pss[ci][:],
                                lhsT=lhsT,
                                rhs=wT[:, t, c0:c1],
                                start=(t == 0),
                                stop=(t == KT - 1),
                            )
                    osb = opool.tile([P, NC], mybir.dt.float16, tag="osb")
                    for ci, (c0, c1) in enumerate(CHUNKS):
                        nc.vector.tensor_copy(out=osb[:, c0:c1], in_=pss[ci][:])
                    m0 = mb * MB + j * P
                    nc.sync.dma_start(out[m0 : m0 + P, :], osb[:])
    return nc


def prep_inputs(x, weight_packed, scales):
    """Host-side shard/layout prep. Returns per-core input maps."""
    x = np.asarray(x)
    weight_packed = np.asarray(weight_packed)
    scales = np.asarray(scales, dtype=np.float16)

    m_total = x.shape[0] * x.shape[1]
    x2d = x.reshape(m_total, K)
    xT = np.ascontiguousarray(x2d.T)  # (K, M) fp16

    wp8 = weight_packed.astype(np.uint8)  # (N, K//2), one byte per int32
    q = np.empty((N, K), dtype=np.uint8)  # unpacked nibbles, natural k order
    q[:, 0::2] = wp8 & 0x0F
    q[:, 1::2] = wp8 >> 4

    in_maps = []
    for c in range(NCORES):
        qTc = q[c * NC : (c + 1) * NC].T  # (K, NC) view
        # re-pair nibbles: byte[k, i] = q[n=i, k] | q[n=i+NHALF, k] << 4
        qT_packed = np.ascontiguousarray(qTc[:, :NHALF] | (qTc[:, NHALF:] << 4))
        sTc = np.ascontiguousarray(scales[c * NC : (c + 1) * NC].T)  # (KG, NC)
        in_maps.append({"xT": xT, "qT": qT_packed, "scalesT": sTc})
    return in_maps


_program_cache: dict[int, bass.Bass] = {}


def get_program(m_total: int = M) -> bass.Bass:
    if m_total not in _program_cache:
        _program_cache[m_total] = build_program(m_total)
    return _program_cache[m_total]


def kernel(x, weight_packed, scales):
    from concourse.bass_utils import run_bass_kernel_spmd

    x = np.asarray(x)
    in_maps = prep_inputs(x, weight_packed, scales)
    res = run_bass_kernel_spmd(get_program(), in_maps, list(range(NCORES)))
    outs = [np.asarray(res.results[c]["out"]) for c in range(NCORES)]
    out2d = np.concatenate(outs, axis=1)  # (M, N) fp16
    return out2d.reshape(x.shape[0], x.shape[1], N)
